# revision 75
# baseline (speedup 1.0000x reference)
"""Block-sparse attention (nn_BlockSparseAttention) on 8 TRN2 NeuronCores.

Strategy: head-parallel (16 heads / 8 cores = 2 heads per core).
Per core, all in bf16 on the TensorEngine with f32 PSUM accumulation:
  1. QKV projections in transposed layout (QT/KT/VT = W^T @ hidden^T),
     q pre-scaled by 1/sqrt(d) on the host, RoPE fused into the
     PSUM->SBUF eviction (partition-shifted PSUM reads).
  2. Attention with scores in [keys, q] orientation so PV needs no
     transposes. Softmax without max-subtraction (scores stay in f32
     exp range). The reference's mask semantics (masked scores = 0 =>
     probs contribution exp(0) = 1) are implemented exactly via a
     decomposition: exp() runs UNMASKED straight out of PSUM on the
     Scalar engine, a cheap bf16 binary mask zeroes unselected blocks,
     and the "+1 per masked key" contributions are restored with two
     tiny extra matmuls per chunk (block-sums of V x complement mask,
     and 64 x complement-mask count for the denominator) accumulated
     into the same PSUM groups. Normalize = single tensor divide.
  3. AllGather of the per-core attention output, chunked along the
     sequence dim (4 collectives) so comm and the column-sharded o_proj
     pipeline with attention of later chunks.
Host side: input rearrangement/casting, top-k block mask, RoPE tables,
and final concat+transpose of the 8 output shards.
"""
import sys

if "/opt/trn_rl_repo" not in sys.path:
    sys.path.insert(0, "/opt/trn_rl_repo")

import numpy as np
import ml_dtypes

import concourse.bass as bass
import concourse.tile as tile
import concourse.mybir as mybir
from concourse import bacc
from concourse.bass_utils import run_bass_kernel_spmd
from concourse.masks import make_identity

# problem constants (hardcoded per harness contract)
B, S, HID = 1, 2048, 2048
NH, HD, BS = 16, 128, 64
RATIO = 0.5
THETA = 10000.0
NCORES = 8
HPC = NH // NCORES          # heads per core = 2
P = 128                     # partitions
CH = HID // P               # contraction chunks = 16
KT = S // P                 # key tiles = 16
FB = 512                    # free-dim block (psum bank)
QC = S // FB                # q chunks = 4
NQB = S // BS               # 32 blocks per side
QB_PER_FB = FB // BS        # 8 q-blocks per 512 chunk
KTB = 2                     # key tiles evicted per ACT/DVE op

BF = mybir.dt.bfloat16
F32 = mybir.dt.float32

_CACHE = {}


def _build():
    nc = bacc.Bacc("TRN2", target_bir_lowering=False, debug=False,
                   num_devices=NCORES)

    hT = nc.dram_tensor("hT", [QC, P, CH, FB], BF, kind="ExternalInput").ap()
    wq = nc.dram_tensor("wq", [HPC, P, CH, P], BF, kind="ExternalInput").ap()
    wk = nc.dram_tensor("wk", [HPC, P, CH, P], BF, kind="ExternalInput").ap()
    wv = nc.dram_tensor("wv", [HPC, P, CH, P], BF, kind="ExternalInput").ap()
    wo = nc.dram_tensor("wo", [P, CH, HPC * P], BF, kind="ExternalInput").ap()
    cosT = nc.dram_tensor("cosT", [P, S], F32, kind="ExternalInput").ap()
    sinT = nc.dram_tensor("sinT", [P, S], F32, kind="ExternalInput").ap()  # pre-signed
    binT = nc.dram_tensor("binT", [P, HPC, KT, NQB], BF, kind="ExternalInput").ap()
    binN = nc.dram_tensor("binN", [NQB, HPC, NQB], BF, kind="ExternalInput").ap()
    out = nc.dram_tensor("out", [HPC * P, S], F32, kind="ExternalOutput").ap()

    with tile.TileContext(nc) as tc:
        with (
            tc.tile_pool(name="cp", bufs=1) as cp,          # persistent tensors
            tc.tile_pool(name="pp", bufs=1, space="PSUM") as pp,
            tc.tile_pool(name="dp", bufs=1, space="DRAM") as dp,
        ):
            QTr = cp.tile([P, HPC, S], BF, name="QTr")
            KTr = cp.tile([P, HPC, S], BF, name="KTr")
            V_sbs = [cp.tile([P, KT, P], BF, name=f"V_h{h}")
                     for h in range(HPC)]
            corrT_sb = cp.tile([NQB, HPC, P], BF, name="corrT_sb")

            # ---------------- QKV + RoPE (phase-scoped pool) ----------------
            qp = tc.alloc_tile_pool(name="qp", bufs=2)
            w_drams = (wq, wk, wv)
            w_sbs = []
            for h in range(HPC):
                for proj in range(3):
                    w_sb = qp.tile([P, CH, P], BF, name="w_sb", tag="w_sb",
                                   bufs=6)
                    if h == 0 and proj <= 1:
                        # first weights on the fast scalar HWDGE queue so the
                        # first QKV chains aren't gated by the slow SWDGE path
                        nc.scalar.dma_start(w_sb[:], w_drams[proj][h])
                    else:
                        nc.gpsimd.dma_start(w_sb[:], w_drams[proj][h])
                    w_sbs.append(w_sb)
            # RoPE tables on the (idle at startup) scalar HWDGE queue: the
            # first eviction needs them almost as early as the first MM
            cos_sb = qp.tile([P, S], F32, name="cos_sb", bufs=1)
            nc.scalar.dma_start(cos_sb[:], cosT[:])
            sin_sb = qp.tile([P, S], F32, name="sin_sb", bufs=1)
            nc.scalar.dma_start(sin_sb[:], sinT[:])
            # hidden^T in 4 s-chunks x 4 c-subtiles; the first QKV matmul
            # only needs the first 512KB subtile
            CSUB = 4
            hT_sbs = []
            for qcb in range(QC):
                subs = []
                for cs in range(CSUB):
                    hT_c = qp.tile([P, CH // CSUB, FB], BF,
                                   name=f"hT_c{qcb}_{cs}", bufs=1)
                    nc.sync.dma_start(
                        hT_c[:], hT[qcb, :, cs * (CH // CSUB):
                                     (cs + 1) * (CH // CSUB), :])
                    subs.append(hT_c)
                hT_sbs.append(subs)
            bin_sb = cp.tile([P, HPC, KT, NQB], BF, name="bin_sb")
            nc.gpsimd.dma_start(bin_sb[:], binT[:])
            binN_sb = cp.tile([NQB, HPC, NQB], BF, name="binN_sb")
            nc.gpsimd.dma_start(binN_sb[:], binN[:])
            wo_sb = cp.tile([P, CH, HPC * P], BF, name="wo_sb")
            nc.gpsimd.dma_start(wo_sb[:], wo[:])
            ones_sb = cp.tile([P, P], BF, name="ones_sb")
            nc.vector.memset(ones_sb[:], 1.0)
            c64_sb = cp.tile([NQB, P], BF, name="c64_sb")
            nc.vector.memset(c64_sb[:], float(BS))
            ident = cp.tile([P, P], BF, name="ident")
            make_identity(nc, ident[:])

            vT_sbs = {}
            for qc in range(QC):
                qsl = slice(qc * FB, (qc + 1) * FB)
                for h in range(HPC):
                    for proj in range(3):
                        w_sb = w_sbs[h * 3 + proj]
                        ps = pp.tile([P, FB], F32, name="ps_acc",
                                     tag="ps_acc", bufs=2)
                        for c in range(CH):
                            nc.tensor.matmul(
                                ps[:],
                                lhsT=w_sb[:, c, :],
                                rhs=hT_sbs[qc][c // (CH // CSUB)][
                                    :, c % (CH // CSUB), :],
                                start=(c == 0),
                                stop=(c == CH - 1),
                            )
                        if proj < 2:
                            dst = QTr if proj == 0 else KTr
                            tcos = qp.tile([P, FB], F32, name="tcos", tag="tcos")
                            nc.vector.tensor_mul(
                                out=tcos[:], in0=ps[:], in1=cos_sb[:, qsl])
                            tsin = qp.tile([P, FB], F32, name="tsin", tag="tsin")
                            nc.vector.tensor_mul(
                                out=tsin[0:64, :], in0=ps[64:128, :],
                                in1=sin_sb[0:64, qsl])
                            nc.vector.tensor_mul(
                                out=tsin[64:128, :], in0=ps[0:64, :],
                                in1=sin_sb[64:128, qsl])
                            nc.vector.tensor_add(
                                out=dst[:, h, qsl], in0=tcos[:], in1=tsin[:])
                        else:
                            vT_c = qp.tile([P, FB], BF, name="vT_c",
                                           tag="vT_c", bufs=4)
                            nc.scalar.copy(out=vT_c[:], in_=ps[:])
                            vT_sbs[(h, qc)] = vT_c
                            # V natural layout via DMA transpose (bf16), on
                            # the scalar queue; block-sums for the mask
                            # correction via a free-dim reduce
                            eng = nc.scalar if h == 0 else nc.sync
                            for j in range(QC):
                                kt = qc * QC + j
                                eng.dma_start(
                                    V_sbs[h][:, kt, :],
                                    vT_c[:, j * P:(j + 1) * P],
                                    transpose=True,
                                )

            # block-sums of V^T -> [d, kb] -> transpose -> corrT [kb, d]
            for h in range(HPC):
                bsum = qp.tile([P, NQB], BF, name="bsum", tag="bsum")
                with nc.allow_low_precision(
                        reason="block-sum correction term, 64-wide bf16 sum"):
                    for qc in range(QC):
                        nc.vector.tensor_reduce(
                            out=bsum[:, qc * QB_PER_FB:(qc + 1) * QB_PER_FB],
                            in_=vT_sbs[(h, qc)].rearrange(
                                "p (b e) -> p b e", e=BS),
                            axis=mybir.AxisListType.X,
                            op=mybir.AluOpType.add,
                        )
                ps_t = pp.tile([NQB, P], BF, name="ps_t", tag="ps_s", bufs=2)
                with nc.allow_low_precision(
                        reason="block-sum correction term, 64-wide bf16 sum"):
                    nc.tensor.transpose(ps_t[:], bsum[:], ident[:])
                nc.scalar.copy(out=corrT_sb[:, h, :], in_=ps_t[:])

            qp.release()

            # ------------- attention + chunked AllGather + o_proj -------------
            wp = tc.alloc_tile_pool(name="wp", bufs=2)
            cc_ins = [dp.tile([HPC * P, FB], BF, name=f"cc_in{qc}")
                      for qc in range(QC - 1)]
            cc_outs = [dp.tile([NCORES * HPC * P, FB], BF, name=f"cc_out{qc}",
                               addr_space="Shared") for qc in range(QC - 1)]
            # last chunk: per-head AGs so o_proj can overlap the final one
            cc_ins_l = [dp.tile([P, FB], BF, name=f"cc_inl{h}")
                        for h in range(HPC)]
            cc_outs_l = [dp.tile([NCORES * P, FB], BF, name=f"cc_outl{h}",
                                 addr_space="Shared") for h in range(HPC)]

            def emit_oproj(qc):
                # o_proj for chunk qc; emitted chunks behind attention so
                # the in-order PE queue never head-of-line blocks on the AG
                ag_sb = wp.tile([P, CH, FB], BF, name="ag_sb", tag="ag_sb")
                nc.sync.dma_start(
                    ag_sb[:], cc_outs[qc].rearrange("(c p) s -> p c s", p=P))
                for strip in range(HPC):
                    ssl = slice(strip * P, (strip + 1) * P)
                    ps_w = pp.tile([P, FB], F32, name="ps_w", tag="ps_acc",
                                   bufs=2)
                    for c in range(CH):
                        nc.tensor.matmul(
                            ps_w[:],
                            lhsT=wo_sb[:, c, ssl],
                            rhs=ag_sb[:, c, :],
                            start=(c == 0),
                            stop=(c == CH - 1),
                        )
                    ot = wp.tile([P, FB], F32, name="ot", tag="ot")
                    nc.vector.tensor_copy(out=ot[:], in_=ps_w[:])
                    nc.sync.dma_start(
                        out[strip * P:(strip + 1) * P, qc * FB:(qc + 1) * FB],
                        ot[:],
                    )

            for qc in range(QC):
                qsl = slice(qc * FB, (qc + 1) * FB)
                qbsl = slice(qc * QB_PER_FB, (qc + 1) * QB_PER_FB)
                for h in range(HPC):
                    if qc == QC - 1:
                        # o_proj lags 3 chunks: the first AllGather absorbs
                        # the cross-core launch skew without blocking PE
                        emit_oproj(h)
                    # Software-pipelined: scores pair k+1 is emitted before
                    # PV/denominator of pair k, so the in-order PE queue has
                    # matmul work while the Scalar engine exps pair k+1.
                    pts = []
                    NP_ = KT // KTB

                    def emit_scores(ktp):
                        ps_s = pp.tile([P, KTB, FB], F32, name="ps_s",
                                       tag="ps_s", bufs=2)
                        for j in range(KTB):
                            kt = KTB * ktp + j
                            nc.tensor.matmul(
                                ps_s[:, j, :],
                                lhsT=KTr[:, h, kt * P:(kt + 1) * P],
                                rhs=QTr[:, h, qsl],
                                start=True, stop=True,
                            )
                        praw = wp.tile([P, KTB, FB], BF, name="praw",
                                       tag="praw", bufs=3)
                        nc.scalar.activation(
                            out=praw[:], in_=ps_s[:],
                            func=mybir.ActivationFunctionType.Exp)
                        pt = wp.tile([P, KTB, FB], BF, name="probsT",
                                     tag="probsT", bufs=2 * NP_)
                        bin_ap = bin_sb[:, h, KTB * ktp:KTB * (ktp + 1), qbsl]
                        nc.vector.tensor_mul(
                            out=pt[:],
                            in0=praw[:],
                            in1=bin_ap[:, :, :, None].to_broadcast(
                                [P, KTB, QB_PER_FB, BS]),
                        )
                        pts.append(pt)

                    for ktp in range(NP_):
                        emit_scores(ktp)
                    ps_o = pp.tile([P, FB], F32, name="ps_o", tag="ps_o", bufs=1)
                    ps_d = pp.tile([P, FB], F32, name="ps_d", tag="ps_d", bufs=1)
                    for kt in range(KT):
                        nc.tensor.matmul(
                            ps_o[:],
                            lhsT=V_sbs[h][:, kt, :],
                            rhs=pts[kt // KTB][:, kt % KTB, :],
                            start=(kt == 0), stop=False,
                        )
                    for kt in range(KT):
                        nc.tensor.matmul(
                            ps_d[:],
                            lhsT=ones_sb[:],
                            rhs=pts[kt // KTB][:, kt % KTB, :],
                            start=(kt == 0), stop=False,
                        )
                    binN_ap = binN_sb[:, h, qbsl]
                    nc.tensor.matmul(
                        ps_o[:],
                        lhsT=corrT_sb[:, h, :],
                        rhs=binN_ap[:, :, None].to_broadcast(
                            [NQB, QB_PER_FB, BS]),
                        start=False, stop=True,
                    )
                    nc.tensor.matmul(
                        ps_d[:],
                        lhsT=c64_sb[:],
                        rhs=binN_ap[:, :, None].to_broadcast(
                            [NQB, QB_PER_FB, BS]),
                        start=False, stop=True,
                    )
                    rden = wp.tile([P, FB], F32, name="rden", tag="rden")
                    nc.vector.reciprocal_approx_fast(out=rden[:], in_=ps_d[:])
                    at_c = wp.tile([P, FB], BF, name="at_c", tag="at_c",
                                   bufs=4)
                    nc.vector.tensor_mul(out=at_c[:], in0=ps_o[:], in1=rden[:])
                    if qc < QC - 1:
                        nc.sync.dma_start(
                            cc_ins[qc][h * P:(h + 1) * P, :], at_c[:])
                    else:
                        nc.sync.dma_start(cc_ins_l[h][:], at_c[:])
                        nc.gpsimd.collective_compute(
                            "AllGather",
                            mybir.AluOpType.bypass,
                            replica_groups=[list(range(NCORES))],
                            ins=[cc_ins_l[h].opt()],
                            outs=[cc_outs_l[h].opt()],
                        )

                if qc < QC - 1:
                    nc.gpsimd.collective_compute(
                        "AllGather",
                        mybir.AluOpType.bypass,
                        replica_groups=[list(range(NCORES))],
                        ins=[cc_ins[qc].opt()],
                        outs=[cc_outs[qc].opt()],
                    )
            emit_oproj(QC - 2)
            # last chunk o_proj: heads arrive via two AGs; read by c-parity
            ags_l = []
            for hh in range(HPC):
                ag_l = wp.tile([P, NCORES, FB], BF, name="ag_l", tag="ag_sb")
                nc.sync.dma_start(
                    ag_l[:],
                    cc_outs_l[hh].rearrange("(b p) s -> p b s", p=P))
                ags_l.append(ag_l)
            qcl = QC - 1
            c_order = [c for c in range(CH) if c % HPC == 0] + \
                      [c for c in range(CH) if c % HPC != 0]
            for strip in range(HPC):
                ssl = slice(strip * P, (strip + 1) * P)
                ps_w = pp.tile([P, FB], F32, name="ps_w", tag="ps_acc", bufs=2)
                for i, c in enumerate(c_order):
                    nc.tensor.matmul(
                        ps_w[:],
                        lhsT=wo_sb[:, c, ssl],
                        rhs=ags_l[c % HPC][:, c // HPC, :],
                        start=(i == 0),
                        stop=(i == CH - 1),
                    )
                ot = wp.tile([P, FB], F32, name="ot", tag="ot")
                nc.vector.tensor_copy(out=ot[:], in_=ps_w[:])
                nc.sync.dma_start(
                    out[strip * P:(strip + 1) * P, qcl * FB:(qcl + 1) * FB],
                    ot[:],
                )
            wp.release()
            del emit_oproj

    nc.compile()
    return nc


def _host_prep(hidden_states, q_w, k_w, v_w, o_w, sparsity_pattern):
    hs = np.asarray(hidden_states, dtype=np.float32).reshape(S, HID)
    qw = np.asarray(q_w, dtype=np.float32)
    kw = np.asarray(k_w, dtype=np.float32)
    vw = np.asarray(v_w, dtype=np.float32)
    ow = np.asarray(o_w, dtype=np.float32)
    sp = np.asarray(sparsity_pattern, dtype=np.float32)

    bf = ml_dtypes.bfloat16

    # hidden^T -> [qcb, p, c, s'] (s-chunk-major so chunk DMAs are contiguous)
    hT = np.ascontiguousarray(
        hs.T.reshape(CH, P, QC, FB).transpose(2, 1, 0, 3)).astype(bf)

    # block mask with per-head top-k threshold
    kk = max(1, int(NH * NQB * NQB * RATIO / NH))
    flat = sp.reshape(NH, -1)
    th = np.partition(flat, -kk, axis=1)[:, -kk]
    bm = (sp > th[:, None, None]).astype(np.float32)  # [NH, 32 qb, 32 kb]

    # RoPE tables in [d, s] layout; sin pre-signed for rotate_half
    inv = 1.0 / (THETA ** (np.arange(0, HD, 2, dtype=np.float32) / HD))
    fr = np.arange(S, dtype=np.float32)[:, None] * inv[None, :]  # [S, 64]
    embT = np.ascontiguousarray(np.concatenate([fr, fr], axis=1).T)  # [128,S]
    cosT = np.cos(embT).astype(np.float32)
    sinT = np.sin(embT).astype(np.float32)
    sinT[:64] *= -1.0

    def w_per_head(w, h, scale=1.0):
        # [HID, 128] -> [p, c, d]
        return np.ascontiguousarray(
            (w[:, h * HD:(h + 1) * HD] * scale)
            .reshape(CH, P, HD).transpose(1, 0, 2))

    qscale = 1.0 / np.sqrt(HD)
    in_maps = []
    for r in range(NCORES):
        heads = [HPC * r + i for i in range(HPC)]
        wq_r = np.stack([w_per_head(qw, h, qscale) for h in heads]).astype(bf)
        wk_r = np.stack([w_per_head(kw, h) for h in heads]).astype(bf)
        wv_r = np.stack([w_per_head(vw, h) for h in heads]).astype(bf)
        wo_r = np.ascontiguousarray(
            ow[:, r * HPC * HD:(r + 1) * HPC * HD]
            .reshape(CH, P, HPC * HD).transpose(1, 0, 2)).astype(bf)
        # bm[h] is [q_block, k_block]; kernel layout wants keys on partitions
        mT = np.stack([
            np.repeat(bm[h].T, BS, axis=0).reshape(KT, P, NQB).transpose(1, 0, 2)
            for h in heads
        ], axis=1)  # [P, HPC, KT, NQB]
        # complement mask [kb, h, qb] for the masked-block corrections
        mN = np.stack([1.0 - bm[h].T for h in heads], axis=1)  # [32, HPC, 32]
        in_maps.append({
            "hT": hT,
            "wq": wq_r, "wk": wk_r, "wv": wv_r, "wo": wo_r,
            "cosT": cosT, "sinT": sinT,
            "binT": np.ascontiguousarray(mT).astype(bf),
            "binN": np.ascontiguousarray(mN).astype(bf),
        })
    return in_maps


def _run(inputs, trace=False, **kwargs):
    if "nc" not in _CACHE:
        _CACHE["nc"] = _build()
    nc = _CACHE["nc"]
    in_maps = _host_prep(**inputs)
    res = run_bass_kernel_spmd(
        nc, in_maps, core_ids=list(range(NCORES)), trace=trace, **kwargs)
    outT = np.empty((HID, S), dtype=np.float32)
    for r in range(NCORES):
        outT[r * HPC * P:(r + 1) * HPC * P] = res.results[r]["out"]
    full = np.ascontiguousarray(outT.T).reshape(B, S, HID)
    return full, res


def kernel(**inputs):
    full, _ = _run(inputs, trace=False)
    return full
